# revision 43
# baseline (speedup 1.0000x reference)
"""Trainium2 Bass kernel for nn_BiLSTMDecoderModel — v10 (final).

v2 base: fused phase A (gather+tanh+projection inside the scan), xp inject
first into split gate-PSUM halves, backward LSTM collapsed to one cell,
deferred decoder DMAs.

v3-v7 (this session, trace-driven):
  * Warm-keeper matmuls: the per-step DVE/ACT serial tail idles the PE long
    enough that the HAM clock gate re-throttles it to 1.2 GHz; filler
    matmuls into a scratch PSUM bank hold it at 2.4 GHz (halves every
    matmul/transpose; k=1 fillers do NOT register as HAM activity, so the
    stationary must be full-k).  Fill is sized per step around the
    phase-A slice PE work and ordering-gated behind the injects so it can
    never delay the chain-critical transposes.
  * Phase-A PE work (transposes + N=256 projection quanta) carries deferred
    ordering deps on the NEXT step's inject so the tile scheduler cannot
    slot it between tanh(h1) and the chain-critical transposes.
  * The DVE cell-update chain (a, tip, v, z', H around the tanh(c)) is the
    latency floor.  DVE ops can read at most one PSUM operand, which
    forbids deeper fusion; measured dead ends: ACT-side tip (slower op,
    gates `v` later), per-half chain split (per-op fixed costs outweigh
    the overlap), removing the pre-tr2 warm-keeper pack (PE FIFO stalls
    at the tanh(h1) wait; +600ns/step).
"""

import sys

sys.path.insert(0, "/opt/trn_rl_repo")

import numpy as np
import ml_dtypes

import concourse.bass as bass
import concourse.mybir as mybir
import concourse.tile as tile
from concourse.tile import add_dep_helper
from concourse import bacc
from concourse.bass_utils import run_bass_kernel_spmd
from concourse.masks import make_identity

import os as _os

DUM1 = int(_os.environ.get("BK_DUM1", "3"))  # warm-keeper MMs before tr2
DUM2 = int(_os.environ.get("BK_DUM2", "5"))  # warm-keeper MMs after inject
VFUSE = bool(int(_os.environ.get("BK_VFUSE", "0")))
FP8 = bool(int(_os.environ.get("BK_FP8", "0")))     # DoubleRow fp8 recurrence
ACTTIP = bool(int(_os.environ.get("BK_ACTTIP", "0")))  # (ti+1) on ACT engine
XS = 64.0  # fp8 weight prescale (compensated by tanh input scale)
V, E, H, NCLS = 100000, 300, 512, 6
B, S = 128, int(_os.environ.get("BK_S", "256"))
NC = 8
BL = B // NC  # 16
G4 = 4 * H  # 2048
G3 = 3 * 2 * H  # 3072
H2 = 2 * H  # 1024

f32 = mybir.dt.float32
fp8 = mybir.dt.float8e4
bf16 = mybir.dt.bfloat16
i32 = mybir.dt.int32
Tanh = mybir.ActivationFunctionType.Tanh
Exp = mybir.ActivationFunctionType.Exp
Ln = mybir.ActivationFunctionType.Ln
Ident = mybir.ActivationFunctionType.Identity
ADD = mybir.AluOpType.add
SUB = mybir.AluOpType.subtract
MUL = mybir.AluOpType.mult
MAX = mybir.AluOpType.max

_cache = {}


def _bf(x):
    return np.ascontiguousarray(x.astype(ml_dtypes.bfloat16))


def _build_program():
    nc = bacc.Bacc(
        "TRN2", target_bir_lowering=False, debug=False, enable_asserts=False,
        num_devices=NC,
    )
    embedW_d = nc.dram_tensor("embedW", [V, E], f32, kind="ExternalInput").ap()
    idx_d = nc.dram_tensor("idx", [128, 32], i32, kind="ExternalInput").ap()
    wihT_d = nc.dram_tensor("wihT", [304, G4], bf16, kind="ExternalInput").ap()
    bwihT_d = nc.dram_tensor("bwihT", [304, G4], bf16, kind="ExternalInput").ap()
    if FP8:
        whhT_d = nc.dram_tensor("whhT", [128, 4 * G4], fp8, kind="ExternalInput").ap()
    else:
        whhT_d = nc.dram_tensor("whhT", [H, G4], bf16, kind="ExternalInput").ap()
    dwhhT_d = nc.dram_tensor("dwhhT", [1028, G3], bf16, kind="ExternalInput").ap()
    dwihT_d = nc.dram_tensor("dwihT", [516, G3], bf16, kind="ExternalInput").ap()
    ecw_d = nc.dram_tensor("ecw", [NCLS, H], f32, kind="ExternalInput").ap()
    clsT_d = nc.dram_tensor("clsT", [1028, 2], bf16, kind="ExternalInput").ap()
    out_d = nc.dram_tensor("out", [NCLS, BL, 2], f32, kind="ExternalOutput").ap()

    with tile.TileContext(nc) as tc:
        _emit(nc, tc, embedW_d, idx_d, wihT_d, bwihT_d, whhT_d, dwhhT_d,
              dwihT_d, ecw_d, clsT_d, out_d)
    nc.compile()
    return nc


def _emit(nc, tc, embedW_d, idx_d, wihT_d, bwihT_d, whhT_d, dwhhT_d, dwihT_d,
          ecw_d, clsT_d, out_d):
    def pool(**kw):
        return tc.alloc_tile_pool(**kw)

    const = pool(name="const", bufs=1)

    # ---- persistent SBUF constants ----
    ident = const.tile([128, 128], f32, tag="ident", name="ident")
    make_identity(nc, ident[:])
    identb = const.tile([128, 128], bf16, tag="identb", name="identb")
    make_identity(nc, identb[:])
    i16 = const.tile([16, 16], bf16, tag="i16", name="i16")
    make_identity(nc, i16[:])
    ones1 = const.tile([1, 16], bf16, tag="ones1", name="ones1")
    nc.gpsimd.memset(ones1[:], 1.0)
    # +1-rider constants: ones448[0:1, c*112:c*112+80] seeds the i/f/o strips
    onesc = const.tile([1, 128], bf16, tag="onesc", name="onesc")
    nc.gpsimd.memset(onesc[:], 1.0)
    ones448 = const.tile([1, 448], bf16, tag="ones448", name="ones448")
    nc.gpsimd.memset(ones448[:], 1.0)
    bias_stat = const.tile([4, 16], bf16, tag="bias_stat", name="bias_stat")
    nc.gpsimd.memset(bias_stat[:], 0.0)
    nc.gpsimd.memset(bias_stat[0:1, :], 1.0)

    def tr(out_ap, in_ap, pin):
        nc.tensor.transpose(out_ap, in_ap, ident[0:pin, 0:pin])

    def trb(out_ap, in_ap, pin):
        nc.tensor.transpose(out_ap, in_ap, identb[0:pin, 0:pin])

    idx_sb = const.tile([128, 32], i32, tag="idx", name="idx")
    nc.sync.dma_start(idx_sb[:], idx_d[:])

    # wih chunk2 rows 0:44 = WihT rows 256:300, row 44 = bias (wihT row 300)
    wih_sb = [const.tile([128, G4], bf16, tag=f"wih{k}", name=f"wih{k}") for k in range(3)]
    bwih_sb = [const.tile([128, G4], bf16, tag=f"bwih{k}", name=f"bwih{k}") for k in range(3)]
    for k in range(2):
        nc.sync.dma_start(wih_sb[k][:], wihT_d[128 * k:128 * (k + 1), :])
        nc.sync.dma_start(bwih_sb[k][:], bwihT_d[128 * k:128 * (k + 1), :])
    nc.sync.dma_start(wih_sb[2][0:45, :], wihT_d[256:301, :])
    nc.sync.dma_start(bwih_sb[2][0:45, :], bwihT_d[256:301, :])

    if FP8:
        whh8 = const.tile([128, 4 * G4], fp8, tag="whh8", name="whh8")
        nc.sync.dma_start(whh8[:], whhT_d[:])
        whh8v = whh8[:].rearrange("p (c n) -> p c n", n=G4)
        whh_sb = None
    else:
        whh_sb = [const.tile([128, G4], bf16, tag=f"whh{k}", name=f"whh{k}") for k in range(4)]
        for k in range(4):
            nc.sync.dma_start(whh_sb[k][:], whhT_d[128 * k:128 * (k + 1), :])

    # backward-cell hidden output (persists into phase D)
    bH = const.tile([128, 64], bf16, tag="bH", name="bH")

    # hidden-state pool allocated deep in the stack — survives into phase D
    pH = pool(name="pH", bufs=2)

    # ======== fused phase-A pools (live through the scan) ========
    pG = pool(name="pG", bufs=2)
    pTh = pool(name="pTh", bufs=2)
    pEm = pool(name="pEm", bufs=2)
    pPst = pool(name="pPst", bufs=1, space="PSUM")
    pPsx = pool(name="pPsx", bufs=1, space="PSUM")
    pXp = pool(name="pXp", bufs=3)
    pXt = pool(name="pXt", bufs=3)

    NW = S * BL // 128  # 32 windows of 8 steps

    awork = {}  # per-window in-flight tiles
    crit = {"dve": None, "pe": None}  # last critical-chain insts of the step

    def _after_tail(inst):
        # keep prefetch copies behind the recurrence-critical DVE chain
        if crit["dve"] is not None:
            add_dep_helper(inst.ins, crit["dve"].ins, sync=False,
                           reason="phase-A copy after step tail")
        return inst

    pending_pe = []  # phase-A PE insts awaiting an ordering dep on the
    # NEXT step's inject (they execute during the next step's windows)

    def _pe_after_tail(inst):
        pending_pe.append(inst)
        return inst

    def _attach_pending_pe():
        if crit["pe"] is not None:
            for inst in pending_pe:
                add_dep_helper(inst.ins, crit["pe"].ins, sync=False,
                               reason="phase-A PE after next-step PE tail")
        pending_pe.clear()

    def a_gather(m):
        g_t = pG.tile([128, 304], f32, tag="gath", name="gath")
        nc.gpsimd.indirect_dma_start(
            out=g_t[:, 0:E],
            out_offset=None,
            in_=embedW_d[:],
            in_offset=bass.IndirectOffsetOnAxis(ap=idx_sb[:, m:m + 1], axis=0),
        )
        awork[m] = {"g": g_t}

    def a_tanh(m):
        st = awork[m]
        th = pTh.tile([128, 304], bf16, tag="th", name="th")
        nc.scalar.activation(th[:, 0:E], st["g"][:, 0:E], Tanh)
        nc.vector.memset(th[:, E:E + 1], 1.0)  # bias rider column
        st["th"] = th

    def a_transpose(m):
        st = awork[m]
        th = st["th"]
        pst = pPst.tile([128, 384], bf16, tag="pst", name="pst")
        _pe_after_tail(nc.tensor.transpose(pst[0:128, 0:128], th[:, 0:128], identb[0:128, 0:128]))
        _pe_after_tail(nc.tensor.transpose(pst[0:128, 128:256], th[:, 128:256], identb[0:128, 0:128]))
        _pe_after_tail(nc.tensor.transpose(pst[0:45, 256:384], th[:, 256:301], identb[0:128, 0:128]))
        st["pst"] = pst

    def a_copy(m):
        st = awork[m]
        pst = st["pst"]
        embT_a = pEm.tile([128, 256], bf16, tag="embTa", name="embTa")
        _after_tail(nc.vector.tensor_copy(embT_a[:], pst[:, 0:256]))
        embT_b = pEm.tile([48, 128], bf16, tag="embTb", name="embTb")
        _after_tail(nc.vector.tensor_copy(embT_b[0:45, :], pst[0:45, 256:384]))
        st["a"] = embT_a
        st["b"] = embT_b
        xp = pXp.tile([128, G4], bf16, tag="xp", name="xp")
        st["xp"] = xp

    def a_proj(m, nb):
        st = awork[m]
        embT_a, embT_b, xp = st["a"], st["b"], st["xp"]
        psx = pPsx.tile([128, 512], f32, tag="psx", name="psx")
        # N=256 quanta so a queued proj MM delays a chain-critical
        # transpose by at most ~110ns
        for hh in range(2):
            cs = 512 * nb + 256 * hh
            ps = psx[:, 256 * hh:256 * (hh + 1)]
            _pe_after_tail(nc.tensor.matmul(
                ps, embT_a[:, 0:128], wih_sb[0][:, cs:cs + 256],
                start=True, stop=False, skip_group_check=True))
            _pe_after_tail(nc.tensor.matmul(
                ps, embT_a[:, 128:256], wih_sb[1][:, cs:cs + 256],
                start=False, stop=False, skip_group_check=True))
            _pe_after_tail(nc.tensor.matmul(
                ps, embT_b[0:45, :], wih_sb[2][0:45, cs:cs + 256],
                start=False, stop=(hh == 1), skip_group_check=True))
        nsl = slice(512 * nb, 512 * (nb + 1))
        if FP8:
            _after_tail(nc.vector.tensor_scalar_mul(xp[:, nsl], psx[:], XS))
        else:
            _after_tail(nc.vector.tensor_copy(xp[:, nsl], psx[:]))
        if nb == 3:
            del st["g"], st["th"], st["pst"]

    A_SLICES = [a_gather, a_tanh, a_transpose, a_copy,
                lambda m: a_proj(m, 0), lambda m: a_proj(m, 1),
                lambda m: a_proj(m, 2), lambda m: a_proj(m, 3)]

    def a_window(m):
        for fn in A_SLICES:
            fn(m)

    def xp_step_copy(t):
        """[16,2048] per-step slice of the window xp ring, at partition 0."""
        w, s = divmod(t, 8)
        xpt = pXt.tile([16, G4], bf16, tag="xpt", name="xpt")
        nc.sync.dma_start(xpt[:], awork[w]["xp"][16 * s:16 * (s + 1), :])
        return xpt

    # ======== pre-loop: windows 0,1 + backward cell ========
    a_window(0)
    a_window(1)

    # -------- backward LSTM single cell on x_0 (h=c=0) --------
    pBw = pool(name="pBw", bufs=1)
    pBps = pool(name="pBps", bufs=1, space="PSUM")
    emb0_a, emb0_b = awork[0]["a"], awork[0]["b"]
    bps = pBps.tile([128, 512], f32, tag="bps", name="bps")
    for j in range(4):
        ns = slice(512 * j, 512 * (j + 1))
        o = bps[32 * j:32 * j + 16, :]
        tp = (0, 32 * j)
        nc.tensor.matmul(o, emb0_a[:, 0:16], bwih_sb[0][:, ns],
                         start=True, stop=False, tile_position=tp,
                         skip_group_check=True)
        nc.tensor.matmul(o, emb0_a[:, 128:144], bwih_sb[1][:, ns],
                         start=False, stop=False, tile_position=tp,
                         skip_group_check=True)
        nc.tensor.matmul(o, emb0_b[0:45, 0:16], bwih_sb[2][0:45, ns],
                         start=False, stop=True, tile_position=tp,
                         skip_group_check=True)
    bT = pBw.tile([128, 512], f32, tag="bT", name="bT")
    nc.scalar.activation(bT[0:112, :], bps[0:112, :], Tanh)
    bpt = pBps.tile([128, 448], f32, tag="bpt", name="bpt")
    for k in range(4):
        tr(bpt[:, 112 * k:112 * (k + 1)], bT[0:112, 128 * k:128 * (k + 1)], 112)
    bv = bpt[:].rearrange("p (c w) -> p c w", w=112)
    btip = pBw.tile([128, 64], f32, tag="btip", name="btip")
    nc.vector.tensor_scalar_add(
        out=btip[:].rearrange("p (c w) -> p c w", w=16),
        in0=bv[:, :, 0:16], scalar1=1.0)
    bzv = pBw.tile([128, 64], f32, tag="bzv", name="bzv")
    zb = bzv[:].rearrange("p (c w) -> p c w", w=16)
    nc.vector.tensor_tensor(
        out=zb, in0=btip[:].rearrange("p (c w) -> p c w", w=16),
        in1=bv[:, :, 64:80], op=MUL)
    btc = pBw.tile([128, 64], f32, tag="btc", name="btc")
    nc.scalar.activation(btc[:], bzv[:], Tanh, scale=0.5)
    nc.vector.scalar_tensor_tensor(
        out=bH[:].rearrange("p (c w) -> p c w", w=16),
        in0=bv[:, :, 96:112], scalar=1.0,
        in1=btc[:].rearrange("p (c w) -> p c w", w=16),
        op0=ADD, op1=MUL)
    pBps.release()
    pBw.release()

    # ======== Phase C: forward scan, 256 steps ========
    pPS0 = pool(name="pPS0", bufs=2, space="PSUM")
    pPS1 = pool(name="pPS1", bufs=2, space="PSUM")
    pT = pool(name="pT", bufs=3)
    pPstT = pool(name="pPstT", bufs=1, space="PSUM")
    pZ = pool(name="pZ", bufs=2)
    pW = pool(name="pW", bufs=3)

    hdt = fp8 if FP8 else bf16
    z_prev = pZ.tile([128, 64], f32, tag="z", name="z")
    H_prev = pH.tile([128, 64], hdt, tag="H", name="H")
    nc.vector.memset(z_prev[:], 0.0)
    nc.vector.memset(H_prev[:], 0.0)

    # warm-keeper: junk matmuls into a dedicated PSUM bank keep the PE HAM
    # clock-gate at 8/8 through the per-step DVE/ACT serial tail.
    pDum = pool(name="pDum", bufs=1, space="PSUM")
    dum_t = pDum.tile([16, 512], f32, tag="dum", name="dum")

    def dummy_mm(n=512, dep=None):
        inst = nc.tensor.matmul(dum_t[:, 0:n], wih_sb[1][:, 0:16],
                                wih_sb[0][:, 0:n], start=True, stop=True,
                                skip_group_check=True)
        if dep is not None:
            add_dep_helper(inst.ins, dep.ins, sync=False,
                           reason="warm-keeper after inject")
        return inst

    HS = 256  # even split: balances ACT serialization (best on HW)

    def inject(t, xpt):
        """New psg bank pair for step t, xp injected (group start)."""
        psg0 = pPS0.tile([128, HS], f32, tag="psg0", name="psg0")
        psg1 = pPS1.tile([128, 512 - HS], f32, tag="psg1", name="psg1")
        for j in range(4):
            nc.tensor.matmul(psg0[32 * j:32 * j + 16, :], i16[:],
                             xpt[:, 512 * j:512 * j + HS],
                             start=True, stop=False, tile_position=(0, 32 * j),
                             skip_group_check=True)
            nc.tensor.matmul(psg1[32 * j:32 * j + 16, :], i16[:],
                             xpt[:, 512 * j + HS:512 * j + 512],
                             start=True, stop=False, tile_position=(0, 32 * j),
                             skip_group_check=True)
        return psg0, psg1

    xpt0 = xp_step_copy(0)
    xpt1 = xp_step_copy(1)
    cur = inject(0, xpt0)
    nxt_xpt = xpt1
    pending_pe.clear()  # preamble windows must not wait on the scan's injects

    for t in range(S):
        psg0, psg1 = cur
        # 1. recurrence matmuls (the serial chain from H_prev)
        if FP8:
            Hv = H_prev[:].rearrange("p (c w) -> p c w", w=16)
            for psgh, c0, cw in ((psg0, 0, HS), (psg1, HS, 512 - HS)):
                for c2 in range(2):
                    for j in range(4):
                        cs = 512 * j + c0
                        nc.tensor.matmul(
                            psgh[32 * j:32 * j + 16, :],
                            Hv[:, 2 * c2:2 * c2 + 2, :],
                            whh8v[:, 2 * c2:2 * c2 + 2, cs:cs + cw],
                            start=False, stop=(c2 == 1),
                            tile_position=(0, 32 * j), skip_group_check=True,
                            perf_mode=mybir.MatmulPerfMode.DoubleRow)
        else:
            for psgh, c0, cw in ((psg0, 0, HS), (psg1, HS, 512 - HS)):
                for kc in range(4):
                    for j in range(4):
                        cs = 512 * j + c0
                        nc.tensor.matmul(
                            psgh[32 * j:32 * j + 16, :],
                            H_prev[:, 16 * kc:16 * (kc + 1)],
                            whh_sb[kc][:, cs:cs + cw],
                            start=False, stop=(kc == 3),
                            tile_position=(0, 32 * j), skip_group_check=True)

        # 2. gate tanh per half (the big half overlaps the tail-half matmuls)
        T_t = pT.tile([128, 512], bf16, tag="T", name="T")
        gsc = 1.0 / XS if FP8 else 1.0
        nc.scalar.activation(T_t[0:112, 0:HS], psg0[0:112, :], Tanh, scale=gsc)
        nc.scalar.activation(T_t[0:112, HS:512], psg1[0:112, :], Tanh, scale=gsc)

        # 3. transposes to hidden-major, interleaved with the t+1 inject so
        # the PE queue is [tr-h0 x2, injA, tr-h1 x2, injB] — the tail
        # transposes aren't stuck behind the full inject.
        pstT = pPstT.tile([128, 448], bf16, tag="pstT", name="pstT")
        for k in range(2):
            trb(pstT[:, 112 * k:112 * (k + 1)], T_t[0:112, 128 * k:128 * (k + 1)], 112)

        # h0-half of `a` and `tip` (2 ops, ~360ns) run on the idle DVE
        # during tanh_h1/tr2/tr3 — they depend only on tr0/tr1 via
        # range-based tile deps and finish well before tr3 (the 4-op
        # version of this overran that window and regressed).
        Tv = pstT[:].rearrange("p (c w) -> p c w", w=112)
        a_t = pW.tile([128, 64], f32, tag="a", name="a")
        tip = pW.tile([128, 64], bf16, tag="tip", name="tip")
        av_all = a_t[:].rearrange("p (c w) -> p c w", w=16)
        tipv_all = tip[:].rearrange("p (c w) -> p c w", w=16)
        zpv_all = z_prev[:].rearrange("p (c w) -> p c w", w=16)
        nc.vector.scalar_tensor_tensor(
            out=av_all[:, 0:2, :], in0=Tv[:, 0:2, 32:48], scalar=1.0,
            in1=zpv_all[:, 0:2, :], op0=ADD, op1=MUL)
        nc.vector.tensor_scalar_add(out=tipv_all[:, 0:2, :],
                                    in0=Tv[:, 0:2, 0:16], scalar1=1.0)
        psg0_n = psg1_n = None
        injA = None
        if t + 1 < S:
            psg0_n = pPS0.tile([128, HS], f32, tag="psg0", name="psg0")
            for j in range(4):
                injA = nc.tensor.matmul(psg0_n[32 * j:32 * j + 16, :], i16[:],
                                 nxt_xpt[:, 512 * j:512 * j + HS],
                                 start=True, stop=False, tile_position=(0, 32 * j),
                                 skip_group_check=True)
        for _ in range(DUM1):
            dummy_mm(128, dep=injA)  # short quanta: tr2 launches within ~75ns
        for k in range(2, 4):
            crit["pe"] = trb(pstT[:, 112 * k:112 * (k + 1)],
                             T_t[0:112, 128 * k:128 * (k + 1)], 112)
        if t + 1 < S:
            psg1_n = pPS1.tile([128, 512 - HS], f32, tag="psg1", name="psg1")
            for j in range(4):
                crit["pe"] = nc.tensor.matmul(
                    psg1_n[32 * j:32 * j + 16, :], i16[:],
                    nxt_xpt[:, 512 * j + HS:512 * j + 512],
                    start=True, stop=False, tile_position=(0, 32 * j),
                    skip_group_check=True)
            cur = (psg0_n, psg1_n)
        _attach_pending_pe()
        # fill-aware warm-keeper: phase-A slices s=2 (3 transposes) and
        # s>=4 (6 proj MMs) already occupy ~0.7us of the post-inject gap
        ns = t % 8  # slice type emitted this step (see section 6)
        nfill = DUM2 if ns in (0, 1, 3) else max(1, DUM2 - 3)
        if t // 8 + 2 >= NW:
            nfill = DUM2  # no slice emitted near the end of the scan
        for _ in range(nfill):
            dummy_mm(384, dep=crit["pe"])

        # 5. chain tail: h1 halves of `a`/`tip` (gated by tr2/tr3), then
        # full v, z', tanh(c), H.  DVE ops may read at most ONE PSUM
        # operand, so every op pairs a pstT (PSUM) view with an SBUF tile.
        nc.vector.scalar_tensor_tensor(
            out=av_all[:, 2:4, :], in0=Tv[:, 2:4, 32:48], scalar=1.0,
            in1=zpv_all[:, 2:4, :], op0=ADD, op1=MUL)
        nc.vector.tensor_scalar_add(out=tipv_all[:, 2:4, :],
                                    in0=Tv[:, 2:4, 0:16], scalar1=1.0)
        v_t = pW.tile([128, 64], bf16, tag="v", name="v")
        vv = v_t[:].rearrange("p (c w) -> p c w", w=16)
        nc.vector.tensor_tensor(out=vv, in0=tipv_all, in1=Tv[:, :, 64:80],
                                op=MUL)
        z_new = pZ.tile([128, 64], f32, tag="z", name="z")
        nc.vector.scalar_tensor_tensor(out=z_new[:], in0=a_t[:], scalar=0.5,
                                       in1=v_t[:], op0=MUL, op1=ADD)
        tc_t = pW.tile([128, 64], bf16, tag="tc", name="tc")
        nc.scalar.activation(tc_t[:], z_new[:], Tanh, scale=0.5)
        H_new = pH.tile([128, 64], hdt, tag="H", name="H")
        crit["dve"] = nc.vector.scalar_tensor_tensor(
            out=H_new[:].rearrange("p (c w) -> p c w", w=16),
            in0=Tv[:, :, 96:112], scalar=1.0,
            in1=tc_t[:].rearrange("p (c w) -> p c w", w=16),
            op0=ADD, op1=MUL)
        z_prev, H_prev = z_new, H_new

        # 6. phase-A slice + next-next xp copy (emitted after the tail so
        # their DVE copies queue behind the critical-chain DVE ops)
        w, s = divmod(t, 8)
        if w + 2 < NW:
            A_SLICES[s](w + 2)
        if t + 2 < S:
            nxt_xpt = xp_step_copy(t + 2)

    pW.release()
    pZ.release()
    if DUM1 + DUM2 > 0:
        dum_rd = pH.tile([16, 512], f32, tag="dumrd", name="dumrd")
        nc.vector.tensor_copy(dum_rd[:], dum_t[:])
    pDum.release()
    pPstT.release()
    pT.release()
    pPS1.release()
    pPS0.release()
    pXt.release()
    pXp.release()
    pPsx.release()
    pPst.release()
    pEm.release()
    pTh.release()
    pG.release()

    # ---- deferred decoder weights (DMA during scan tail) ----
    dwhh_sb = [const.tile([128, G3], bf16, tag=f"dwhh{k}", name=f"dwhh{k}") for k in range(9)]
    for k in range(8):
        nc.sync.dma_start(dwhh_sb[k][:], dwhhT_d[128 * k:128 * (k + 1), :])
    nc.sync.dma_start(dwhh_sb[8][0:4, :], dwhhT_d[1024:1028, :])

    dwih_sb = [const.tile([128, G3], bf16, tag=f"dwih{k}", name=f"dwih{k}") for k in range(5)]
    for k in range(4):
        nc.sync.dma_start(dwih_sb[k][:], dwihT_d[128 * k:128 * (k + 1), :])
    nc.sync.dma_start(dwih_sb[4][0:4, :], dwihT_d[512:516, :])

    cls_sb = [const.tile([128, 2], bf16, tag=f"cls{k}", name=f"cls{k}") for k in range(9)]
    for k in range(8):
        nc.sync.dma_start(cls_sb[k][:], clsT_d[128 * k:128 * (k + 1), :])
    nc.sync.dma_start(cls_sb[8][0:4, :], clsT_d[1024:1028, :])

    # ======== Phase D: decoder (6 GRU steps + logits + log_softmax) ========
    pD = pool(name="pD", bufs=1)
    pDgi = pool(name="pDgi", bufs=1, space="PSUM")

    ce_t = pD.tile([NCLS, H], f32, tag="ce", name="ce")
    nc.sync.dma_start(ce_t[:], ecw_d[:])
    ce2 = pD.tile([NCLS, H], f32, tag="ce2", name="ce2")
    nc.scalar.activation(ce2[:], ce_t[:], Tanh)
    psc = pDgi.tile([128, 24], f32, tag="psc", name="psc")
    for k in range(4):
        tr(psc[:, 6 * k:6 * (k + 1)], ce2[0:NCLS, 128 * k:128 * (k + 1)], NCLS)
    ceT = pD.tile([128, 24], bf16, tag="ceT", name="ceT")
    nc.vector.tensor_copy(ceT[:], psc[:])

    psgi = pDgi.tile([NCLS, G3], f32, tag="psgi", name="psgi")
    for ng in range(6):
        ns = slice(512 * ng, 512 * (ng + 1))
        for kc in range(4):
            nc.tensor.matmul(psgi[:, ns], ceT[:, 6 * kc:6 * (kc + 1)],
                             dwih_sb[kc][:, ns], start=(kc == 0), stop=False)
        nc.tensor.matmul(psgi[:, ns], bias_stat[0:4, 0:NCLS],
                         dwih_sb[4][0:4, ns], start=False, stop=True)
    gi_sb = pD.tile([NCLS, G3], bf16, tag="gi", name="gi")
    nc.scalar.activation(gi_sb[:], psgi[:], Ident)

    psgT = pDgi.tile([128, 48], bf16, tag="psgT", name="psgT")
    for gc in range(8):
        nc.tensor.transpose(psgT[:, 6 * gc:6 * (gc + 1)], gi_sb[0:NCLS, 2048 + 128 * gc:2048 + 128 * (gc + 1)], identb[0:NCLS, 0:NCLS])
    giT = pD.tile([128, 48], f32, tag="giT", name="giT")
    nc.vector.tensor_copy(giT[:], psgT[:])
    gi_row = pD.tile([1, NCLS * G3], bf16, tag="girow", name="girow")
    for c in range(NCLS):
        nc.sync.dma_start(gi_row[0:1, G3 * c:G3 * (c + 1)], gi_sb[c:c + 1, :])
    pDgi.release()
    pDps = pool(name="pDps", bufs=1, space="PSUM")

    Hd = pD.tile([128, 128], bf16, tag="Hd", name="Hd")
    nc.vector.tensor_scalar_mul(Hd[:, 0:64], H_prev[:], 0.5)
    nc.vector.tensor_scalar_mul(Hd[:, 64:128], bH[:], 0.5)

    l_all = pD.tile([16, 12], f32, tag="lall", name="lall")

    for c in range(NCLS):
        psd0 = pDps.tile([128, 512], f32, tag="psd0", name="psd0")
        psd1 = pDps.tile([128, 512], f32, tag="psd1", name="psd1")
        # r,z gates first so their tanh/transposes overlap the n-gate matmuls
        for kc in range(8):
            lh = Hd[:, 16 * kc:16 * (kc + 1)]
            for ng in range(4):
                nc.tensor.matmul(
                    psd0[32 * ng:32 * ng + 16, :], lh,
                    dwhh_sb[kc][:, 512 * ng:512 * (ng + 1)],
                    start=(kc == 0), stop=False, tile_position=(0, 32 * ng),
                    skip_group_check=True)
        for ng in range(4):
            nc.tensor.matmul(
                psd0[32 * ng:32 * ng + 16, :], ones1[:],
                gi_row[0:1, G3 * c + 512 * ng:G3 * c + 512 * (ng + 1)],
                start=False, stop=True, tile_position=(0, 32 * ng),
                skip_group_check=True)
        Trz = pD.tile([128, 512], bf16, tag="Trz", name="Trz")
        nc.scalar.activation(Trz[0:112, :], psd0[0:112, :], Tanh)
        for kc in range(8):
            lh = Hd[:, 16 * kc:16 * (kc + 1)]
            for ng in (4, 5):
                nc.tensor.matmul(
                    psd1[32 * (ng - 4):32 * (ng - 4) + 16, :], lh,
                    dwhh_sb[kc][:, 512 * ng:512 * (ng + 1)],
                    start=(kc == 0), stop=False, tile_position=(0, 32 * (ng - 4)),
                    skip_group_check=True)
        for ng in (4, 5):
            j = ng - 4
            nc.tensor.matmul(
                psd1[32 * j:32 * j + 16, :], bias_stat[0:4, :],
                dwhh_sb[8][0:4, 512 * ng:512 * (ng + 1)],
                start=False, stop=True, tile_position=(0, 32 * j),
                skip_group_check=True)
        pstz = pDps.tile([128, 448], bf16, tag="pstz", name="pstz")
        for k in range(4):
            trb(pstz[:, 112 * k:112 * (k + 1)], Trz[0:112, 128 * k:128 * (k + 1)], 112)
        hn_sb = pD.tile([48, 512], bf16, tag="hn", name="hn")
        nc.vector.tensor_copy(hn_sb[:], psd1[0:48, :])
        psn = pDps.tile([128, 192], bf16, tag="psn", name="psn")
        for k in range(4):
            trb(psn[:, 48 * k:48 * (k + 1)], hn_sb[0:48, 128 * k:128 * (k + 1)], 48)

        zv = pstz[:].rearrange("p (c w) -> p c w", w=112)
        nv = psn[:].rearrange("p (c w) -> p c w", w=48)
        trp = pD.tile([128, 128], f32, tag="trp", name="trp")
        trpv = trp[:].rearrange("p (g w) -> p g w", w=16)
        sn_t = pD.tile([128, 128], f32, tag="sn", name="sn")
        snv = sn_t[:].rearrange("p (g w) -> p g w", w=16)
        nT = pD.tile([128, 128], bf16, tag="nT", name="nT")
        for s in range(2):
            nc.vector.tensor_scalar_add(
                out=trpv[:, 4 * s:4 * s + 4, :],
                in0=zv[:, :, 32 * s:32 * s + 16], scalar1=1.0)
            nc.vector.tensor_tensor(
                out=snv[:, 4 * s:4 * s + 4, :],
                in0=trpv[:, 4 * s:4 * s + 4, :],
                in1=nv[:, :, 32 * s:32 * s + 16], op=MUL)
        sb_t = pD.tile([128, 128], f32, tag="sb", name="sb")
        sbv = sb_t[:].rearrange("p (g w) -> p g w", w=16)
        giTb = giT[:].rearrange("p (g c) -> p g c", c=6)[:, :, c:c + 1].to_broadcast([128, 8, 16])
        nc.vector.scalar_tensor_tensor(out=sbv, in0=snv, scalar=0.5,
                                       in1=giTb, op0=MUL, op1=ADD)
        nc.scalar.activation(nT[:], sb_t[:], Tanh)
        d_t = pD.tile([128, 128], f32, tag="dt", name="dt")
        nc.vector.tensor_tensor(out=d_t[:], in0=Hd[:], in1=nT[:], op=SUB)
        e_t = pD.tile([128, 128], f32, tag="et", name="et")
        ev = e_t[:].rearrange("p (g w) -> p g w", w=16)
        dv = d_t[:].rearrange("p (g w) -> p g w", w=16)
        for s in range(2):
            nc.vector.scalar_tensor_tensor(
                out=ev[:, 4 * s:4 * s + 4, :],
                in0=zv[:, :, 64 + 32 * s:80 + 32 * s], scalar=1.0,
                in1=dv[:, 4 * s:4 * s + 4, :], op0=ADD, op1=MUL)
        hn2 = pD.tile([128, 128], f32, tag="hn2", name="hn2")
        nc.vector.scalar_tensor_tensor(out=hn2[:], in0=e_t[:], scalar=0.5,
                                       in1=nT[:], op0=MUL, op1=ADD)
        Hd_new = pD.tile([128, 128], bf16, tag="Hd", name="Hd")
        nc.scalar.activation(Hd_new[:], hn2[:], Tanh)

        psl = pDps.tile([16, 2], f32, tag="psl", name="psl")
        for kc in range(8):
            nc.tensor.matmul(psl[:], Hd_new[:, 16 * kc:16 * (kc + 1)],
                             cls_sb[kc][:, 0:2], start=(kc == 0), stop=False)
        nc.tensor.matmul(psl[:], bias_stat[0:4, :], cls_sb[8][0:4, 0:2],
                         start=False, stop=True)
        nc.vector.tensor_copy(l_all[:, 2 * c:2 * c + 2], psl[:])
        Hd = Hd_new

    la = l_all[:].rearrange("p (c t) -> p c t", t=2)
    mx = pD.tile([16, 6], f32, tag="mx", name="mx")
    nc.vector.tensor_tensor(out=mx[:].rearrange("p (c o) -> p c o", o=1),
                            in0=la[:, :, 0:1], in1=la[:, :, 1:2], op=MAX)
    d0 = pD.tile([16, 12], f32, tag="d0", name="d0")
    d0v = d0[:].rearrange("p (c t) -> p c t", t=2)
    mxb = mx[:].rearrange("p (c o) -> p c o", o=1).to_broadcast([16, 6, 2])
    nc.vector.tensor_tensor(out=d0v, in0=la, in1=mxb, op=SUB)
    ex = pD.tile([16, 12], f32, tag="ex", name="ex")
    nc.scalar.activation(ex[:], d0[:], Exp)
    se = pD.tile([16, 6], f32, tag="se", name="se")
    nc.vector.tensor_reduce(out=se[:].rearrange("p (c o) -> p c o", o=1),
                            in_=ex[:].rearrange("p (c t) -> p c t", t=2),
                            op=ADD, axis=mybir.AxisListType.X)
    ls = pD.tile([16, 6], f32, tag="ls", name="ls")
    nc.scalar.activation(ls[:], se[:], Ln)
    ov = pD.tile([16, 12], f32, tag="ov", name="ov")
    lsb = ls[:].rearrange("p (c o) -> p c o", o=1).to_broadcast([16, 6, 2])
    nc.vector.tensor_tensor(out=ov[:].rearrange("p (c t) -> p c t", t=2),
                            in0=d0v, in1=lsb, op=SUB)
    nc.sync.dma_start(out_d[:].rearrange("c b t -> b c t"),
                      ov[:].rearrange("p (c t) -> p c t", t=2))

    pDps.release()
    pD.release()
    pH.release()
    const.release()


def _prep_inputs(seq, classes, embed_W, embed_class_W, f_Wih, f_Whh, f_b,
                 b_Wih, b_Whh, b_b, d_Wih, d_Whh, d_bih, d_bhh, cls_W, cls_b):
    seq = np.asarray(seq)
    s4 = np.concatenate([np.full(H, 0.5), np.full(H, 0.5), np.ones(H),
                         np.full(H, 0.5)]).astype(np.float32)
    s3 = np.concatenate([np.full(H2, 0.5), np.full(H2, 0.5),
                         np.ones(H2)]).astype(np.float32)

    def padrows(a, rows):
        out = np.zeros((rows, a.shape[1]), np.float32)
        out[:a.shape[0]] = a
        return out

    wihT = padrows(np.concatenate(
        [(f_Wih * s4[:, None]).T, (f_b * s4)[None, :]], axis=0), 304)
    bwihT = padrows(np.concatenate(
        [(b_Wih * s4[:, None]).T, (b_b * s4)[None, :]], axis=0), 304)
    whhT = ((f_Whh * s4[:, None]) * 0.5).T.astype(np.float32)
    if FP8:
        whh_arr = np.ascontiguousarray(
            (whhT * XS).reshape(512 // 128, 128, 4 * H).transpose(1, 0, 2)
            .reshape(128, 16 * H)).astype(ml_dtypes.float8_e4m3)
    else:
        whh_arr = _bf(whhT)
    dwhhT = padrows(np.concatenate(
        [(d_Whh * s3[:, None]).T, (d_bhh * s3)[None, :]], axis=0), 1028)
    dbi = (np.asarray(d_bih, np.float32) * s3).astype(np.float32)
    dbi[:2 * H2] += (np.asarray(d_bhh, np.float32) * s3)[:2 * H2]
    dwihT = padrows(np.concatenate(
        [(d_Wih * s3[:, None]).T, dbi[None, :]], axis=0), 516)
    clsT = padrows(np.concatenate(
        [np.asarray(cls_W, np.float32).T, np.asarray(cls_b, np.float32)[None, :]],
        axis=0), 1028)
    ecw = np.asarray(embed_class_W, np.float32)[np.asarray(classes)]

    shared = {
        "embedW": np.ascontiguousarray(np.asarray(embed_W, np.float32)),
        "wihT": _bf(wihT), "bwihT": _bf(bwihT), "whhT": whh_arr,
        "dwhhT": _bf(dwhhT), "dwihT": _bf(dwihT),
        "ecw": np.ascontiguousarray(ecw),
        "clsT": _bf(clsT),
    }
    in_maps = []
    for c in range(NC):
        tok = np.asarray(seq[BL * c:BL * (c + 1), :], np.int32)  # [16, 256]
        idx = np.ascontiguousarray(
            tok.T.reshape(S * BL).reshape(32, 128).T.astype(np.int32))
        m = dict(shared)
        m["idx"] = idx
        in_maps.append(m)
    return in_maps


def kernel(**inputs):
    if "nc" not in _cache:
        _cache["nc"] = _build_program()
    nc = _cache["nc"]
    in_maps = _prep_inputs(**inputs)
    import os
    trace = bool(int(os.environ.get("BK_TRACE", "0")))
    res = run_bass_kernel_spmd(nc, in_maps, core_ids=list(range(NC)),
                               trace=trace)
    _cache["last_result"] = res
    outs = [res.results[c]["out"] for c in range(NC)]
    return np.concatenate(outs, axis=1).astype(np.float32)

